# revision 20
# baseline (speedup 1.0000x reference)
"""Trainium2 Bass kernel for nn_FactorMask (9-tap masked-stencil op).

Contract: kernel(**inputs) takes FULL inputs (inp [8,224,224,32] f32,
kernel [9,1,1,1,32], mask [9,1,1,1,32]) and returns the FULL output
[8,224,224,32] f32. Internally: batch-parallel across 8 NeuronCores
(1 image per core), host-side repack to a channels-on-partition layout,
Bass/Tile kernel per core, host-side gather.

Layout per core: xh [128, 58, 226] fp16 where partition p = 32*q + c
(q = H-quarter 0..3, c = channel), rows = 58 padded rows of that quarter
(56 output rows + 1 halo row each side), cols = 226 zero-padded W.
All 9 stencil taps are then pure free-dim AP offsets.

Math (per pixel/channel): A_e = |m_e x_e - k_e|, mu = M/9 - kbar with
M = sum m_e x_e, norm1 = sum A_e, S = sum min(A_e, mu),
    var = norm1 + 9 mu - 2 S
    out = (1 - var/9)(1 - norm1/9) = W * (W + kbar - F/9)
with W = (9 - norm1)/9 and F = M - 2S.

v4 structure (per segment-pair of 2x2 rows):
- PE bank MF: 9 mean-conv matmuls (diag m_e) -> Act folds mu; later the
  9 min-field matmuls (-2I) CONTINUE accumulating into the same bank
  (start=False) so F = M - 2S costs no extra pass.
- PE bank N1: identity matmuls over A fields -> Act folds W = 1 - N1/9.
- Fused final: one custom DVE op out = Src0*((Src0+C0) - Src1*C1) with
  Src0 = W (SBUF fp16), Src1 = F (PSUM), C0 = kbar, C1 = 1/9 -- replaces
  the old STT + 2xTT chain (DVE instructions can read only ONE PSUM
  operand, so W must come via the Act fold).
"""

import os
import sys

for _p in ("/opt/trn_rl_repo", "/opt/pypackages"):
    if _p not in sys.path:
        sys.path.insert(0, _p)

import numpy as np

import concourse.bacc as bacc
import concourse.mybir as mybir
import concourse.tile as tile
from concourse.bass_utils import run_bass_kernel_spmd

# ---- problem constants (hardcoded per the task spec) ----
B, H, W, C = 8, 224, 224, 32
E = 9  # taps
NCORES = 8
Q = 4  # H-quarters per image -> 4*32 = 128 partitions
RQ = H // Q  # 56 output rows per quarter
RA = RQ + 2  # 58 rows incl halo
WP = W + 2  # 226 padded cols
P = 128

# tap order must match reference: element 0 = center, then (y,x) raster
# order skipping center, with shift (dy,dx) = (y-1, x-1)
TAPS = [(0, 0)] + [
    (dy, dx) for dy in (-1, 0, 1) for dx in (-1, 0, 1) if not (dy == 0 and dx == 0)
]

# ---- tunables ----
SEG = 2  # rows per matmul segment (2*224 = 448 <= 512)
NDVE = int(os.environ.get("FM_NDVE", "2"))  # taps whose A is made on DVE
NPAIR_A = int(os.environ.get("FM_NPAIR_A", "1"))  # DVE pre-sum pairs (norm1)
NPAIR_S = int(os.environ.get("FM_NPAIR_S", "4"))  # DVE pre-sum pairs (S)
F32 = mybir.dt.float32
HDT = mybir.dt.float16
U16 = mybir.dt.uint16

_CACHE = {}


def _register_fm_final():
    import concourse.dve_ops as DOPS
    from concourse.dve_spec import Spec, Src0, Src1, C0, C1, lower

    name = "FM_FINAL_ANT"
    if name in DOPS._SUB_OPCODE_FOR_NAME:
        for op in DOPS.OPS:
            if op.name == name:
                return op
    spec = Spec(
        body=Src0 * ((Src0 + C0) - Src1 * C1),
        reference=lambda in0, in1, s0, s1, imm2: (
            in0 * ((in0 + s0) - in1 * s1)
        ),
    )
    row = max(DOPS._SUB_OPCODE_FOR_NAME.values()) + 1
    assert row < 0x20
    DOPS._SUB_OPCODE_FOR_NAME[name] = row
    shas = {}
    for ver in ("v3", "v4"):
        shas[ver] = DOPS.DveOpSpec(
            name=name, opcode=row, uops=lower(spec, ver=ver), rd1_en=True
        ).sha(ver)
    op = DOPS.DveOp(name, spec, subdim=False, uops_sha=shas)
    DOPS.OPS.append(op)
    DOPS.CUSTOM_DVE_SPECS[name] = spec
    return op


def _build_program():
    nc = bacc.Bacc(
        "TRN2", target_bir_lowering=False, debug=False, num_devices=NCORES
    )
    xh_d = nc.dram_tensor("xh", [P, RA, WP], HDT, kind="ExternalInput").ap()
    # pv columns: [0:9]=m_e scale, [9:18]=-k_e bias, [18]=-kbar, [19]=+kbar
    pv_d = nc.dram_tensor("pv", [P, 2 * E + 2], F32, kind="ExternalInput").ap()
    # wm[:, e, :] = diag(m_e) e<9 ; wm[:, 9, :] = I ; wm[:, 10, :] = -2I
    wm_d = nc.dram_tensor("wm", [P, E + 2, P], HDT, kind="ExternalInput").ap()
    y_d = nc.dram_tensor("y", [P, RQ, W], HDT, kind="ExternalOutput").ap()

    fm_final = _register_fm_final()
    with tile.TileContext(nc) as tc:
        _emit(tc, nc, xh_d, pv_d, wm_d, y_d, fm_final)
    nc.compile()
    return nc


def _band_sizes():
    sched = os.environ.get("FM_BANDS", "4,8,8,8,8,8,8,4")
    sizes = [int(s) for s in sched.split(",")]
    assert sum(sizes) == RQ and all(s % (2 * SEG) == 0 for s in sizes)
    return sizes


def _emit(tc, nc, xh_d, pv_d, wm_d, y_d, fm_final):
    Abs = mybir.ActivationFunctionType.Abs
    Ident = mybir.ActivationFunctionType.Identity
    Copy = mybir.ActivationFunctionType.Copy
    mult = mybir.AluOpType.mult
    add = mybir.AluOpType.add
    amin = mybir.AluOpType.min
    band_ = mybir.AluOpType.bitwise_and

    def band_cfg(band, nbands):
        # late bands: DVE is the endgame bottleneck (mins/finals with no Act
        # work left) -> fewer DVE pre-sum pairs, taps on Act.
        tail = int(os.environ.get("FM_TAILBANDS", "2"))
        if band >= nbands - tail:
            return (
                int(os.environ.get("FM_TNDVE", "0")),
                int(os.environ.get("FM_TNPAIR_A", "1")),
                int(os.environ.get("FM_TNPAIR_S", "0")),
            )
        return (NDVE, NPAIR_A, NPAIR_S)

    with (
        tc.tile_pool(name="const", bufs=1) as cpool,
        tc.tile_pool(name="xin", bufs=int(os.environ.get("FM_XBUFS", "3"))) as xpool,
        tc.tile_pool(name="work", bufs=2) as wpool,
        tc.tile_pool(name="mf", bufs=int(os.environ.get("FM_MFBUFS","6")), space="PSUM") as mfpool,
        tc.tile_pool(name="n1", bufs=int(os.environ.get("FM_N1BUFS","2")), space="PSUM") as n1pool,
    ):
        # Startup DMA issue order: first matmul gates on wm + band-0 xh, so
        # those two go first; pv (tiny, needed by Act) issues third.
        sizes = _band_sizes()
        # spread startup DMAs across queues so they trigger in parallel as
        # soon as each engine's preamble finishes
        xbh0 = xpool.tile([P, sizes[0] + 2, WP], HDT, tag="xbh")
        nc.sync.dma_start(xbh0[:], xh_d[:, 0 : sizes[0] + 2, :])
        wm = cpool.tile([P, E + 2, P], HDT)
        nc.gpsimd.dma_start(wm[:], wm_d[:])
        pv = cpool.tile([P, 2 * E + 2], F32)
        nc.scalar.dma_start(pv[:], pv_d[:])

        # PE p-state warmup: the Tensor engine ramps 0.65->2.4GHz over ~3us
        # of continuous work. Run throwaway matmuls on a memset tile during
        # the startup DMA wait so real matmuls start at full clock.
        nwarm = int(os.environ.get("FM_WARM", "16"))
        if nwarm:
            warm = cpool.tile([P, 512], HDT)
            nc.vector.memset(warm[:], 0.5)
            wp = n1pool.tile([P, SEG * W], F32, tag="n1")
            for _ in range(nwarm):
                nc.tensor.matmul(
                    wp[:], warm[:, 0:P], warm[:, 0 : SEG * W], start=True, stop=True
                )

        r0 = 0
        for band, rb in enumerate(sizes):
            ndve_b, npa_b, nps_b = band_cfg(band, len(sizes))
            a_pairs = [(1 + 2 * i, 2 + 2 * i) for i in range(npa_b)]
            s_pairs = [(1 + 2 * i, 2 + 2 * i) for i in range(nps_b)]
            a_paired = {t for p in a_pairs for t in p}
            s_paired = {t for p in s_pairs for t in p}
            nseg = rb // SEG
            npair = nseg // 2
            if band == 0:
                xbh = xbh0
            else:
                xbh = xpool.tile([P, rb + 2, WP], HDT, tag="xbh")
                nc.sync.dma_start(xbh[:], xh_d[:, r0 : r0 + rb + 2, :])

            # A_e = |m_e * x - k_e| on the tap's shifted window. The last
            # NDVE taps are made on DVE (two 4x tensor_scalar ops).
            A = [None] * E

            def emit_a(e):
                dy, dx = TAPS[e]
                a = wpool.tile(
                    [P, rb, W], HDT, tag=f"A{e}",
                    bufs=3 if e < int(os.environ.get("FM_ADEEP", "7")) else 2,
                )
                win = xbh[:, 1 + dy : 1 + dy + rb, 1 + dx : 1 + dx + W]
                if e < E - ndve_b:
                    nc.scalar.activation(
                        a[:],
                        win,
                        Abs,
                        bias=pv[:, E + e : E + e + 1],
                        scale=pv[:, e : e + 1],
                    )
                else:
                    nc.vector.tensor_scalar(
                        a[:],
                        win,
                        pv[:, e : e + 1],
                        pv[:, E + e : E + e + 1],
                        mult,
                        add,
                    )
                    nc.vector.tensor_scalar(
                        a[:].bitcast(U16), a[:].bitcast(U16), 0x7FFF, None, band_
                    )
                A[e] = a

            # First Act A-fields go ahead of the mu-folds in the Act queue:
            # the mu-folds wait on PE chains, and an in-order Act queue would
            # otherwise idle at band start with ready A-work behind them.
            n_early = int(os.environ.get("FM_AEARLY", "3"))
            for e in range(min(n_early, E - ndve_b)):
                emit_a(e)

            # mean conv per segment into 1-bank MF tiles (deep pipelining);
            # mu = MF/9 - kbar folded per segment on Act.
            mean = wpool.tile(
                [P, rb, W], HDT, tag="mean",
                bufs=int(os.environ.get("FM_MBUFS", "2")),
            )
            mfs = []
            for s in range(nseg):
                mf = mfpool.tile([P, SEG * W], F32, tag="mf")
                mfs.append(mf)
                i0 = s * SEG
                for e, (dy, dx) in enumerate(TAPS):
                    rhs = xbh[
                        :, i0 + 1 + dy : i0 + 1 + dy + SEG, 1 + dx : 1 + dx + W
                    ]
                    nc.tensor.matmul(
                        mf[:],
                        wm[:, e, :],
                        rhs,
                        start=(e == 0),
                        stop=(e == E - 1),
                    )
                nc.scalar.activation(
                    mean[:, i0 : i0 + SEG, :],
                    mf[:],
                    Ident,
                    bias=pv[:, 2 * E : 2 * E + 1],
                    scale=1.0 / E,
                )

            for e in range(E):
                if A[e] is None:
                    emit_a(e)

            # optional DVE pre-sums of A pairs (frees PE passes)
            APs = {}
            for i, (t0, t1) in enumerate(a_pairs):
                pa = wpool.tile([P, rb, W], HDT, tag=f"PA{i}")
                nc.vector.tensor_tensor(pa[:], A[t0][:], A[t1][:], add)
                APs[(t0, t1)] = pa
            norm_fields = [A[0]] + [APs[p] for p in a_pairs] + [
                A[e] for e in range(1, E) if e not in a_paired
            ]

            # norm1 per segment (1-bank tiles, fine-grained pipelining);
            # W = (9 - norm1)/9 folded per segment on Act.
            SW = SEG * W
            w9 = wpool.tile([P, rb, W], HDT, tag="w9")
            for s in range(nseg):
                n1 = n1pool.tile([P, SW], F32, tag="n1")
                i0 = s * SEG
                for jf, f in enumerate(norm_fields):
                    nc.tensor.matmul(
                        n1[:],
                        wm[:, E, :],
                        f[:, i0 : i0 + SEG, :],
                        start=(jf == 0),
                        stop=(jf == len(norm_fields) - 1),
                    )
                nc.scalar.activation(
                    w9[:, i0 : i0 + SEG, :],
                    n1[:],
                    Copy,
                    bias=1.0,
                    scale=-1.0 / E,
                )

            # min-fields sm_e = min(A_e, mean)  (one 2x TT per tap).
            # FM_INPLACE=1 overwrites A_e (saves SBUF for bigger bands; the
            # TT then waits for the N1 matmuls that consume A_e).
            inplace = bool(int(os.environ.get("FM_INPLACE", "0")))
            sm = []
            for e in range(E):
                if inplace:
                    t = A[e]
                else:
                    t = wpool.tile(
                        [P, rb, W],
                        HDT,
                        tag=f"sm{e}",
                        bufs=int(os.environ.get("FM_SMBUFS", "2")),
                    )
                nc.vector.tensor_tensor(t[:], A[e][:], mean[:], amin)
                sm.append(t)

            # in-place pair pre-sums: sm[t0] += sm[t1]
            for t0, t1 in s_pairs:
                nc.vector.tensor_tensor(sm[t0][:], sm[t0][:], sm[t1][:], add)
            s_fields = [sm[0]] + [sm[t0] for t0, _ in s_pairs] + [
                sm[e] for e in range(1, E) if e not in s_paired
            ]

            # F = M - 2S: min-field matmuls (-2I) continue accumulating into
            # the closed MF group (start=False); then the fused final per
            # segment: out = W * ((W + kbar) - F/9)  (one custom DVE op).
            ob = wpool.tile([P, rb, W], HDT, tag="ob")
            for s in range(nseg):
                mf = mfs[s]
                i0 = s * SEG
                for jf, f in enumerate(s_fields):
                    nc.tensor.matmul(
                        mf[:],
                        wm[:, E + 1, :],
                        f[:, i0 : i0 + SEG, :],
                        start=False,
                        stop=(jf == len(s_fields) - 1),
                        skip_group_check=True,
                    )
                i1 = i0 + SEG
                nc.vector._custom_dve(
                    fm_final,
                    out=ob[:, i0:i1, :],
                    in0=w9[:, i0:i1, :],
                    in1=mf[:],
                    s0=pv[:, 2 * E + 1 : 2 * E + 2],
                    s1=1.0 / E,
                )
            nc.sync.dma_start(y_d[:, r0 : r0 + rb, :], ob[:])
            r0 += rb


def _host_pack(inp, kern, mask):
    """Build per-core input maps."""
    inp = np.ascontiguousarray(inp, dtype=np.float32)
    kern = np.asarray(kern, dtype=np.float32).reshape(E, C)
    mask = np.asarray(mask, dtype=np.float32).reshape(E, C)

    m = np.abs(mask) / (np.abs(mask).max() + np.float32(1e-6))  # [E,C]
    kbar = kern.mean(axis=0)  # [C]

    cidx = np.arange(P) % C
    pv = np.empty((P, 2 * E + 2), np.float32)
    for e in range(E):
        pv[:, e] = m[e][cidx]
        pv[:, E + e] = -kern[e][cidx]
    pv[:, 2 * E] = -kbar[cidx]
    pv[:, 2 * E + 1] = kbar[cidx]

    wm = np.zeros((P, E + 2, P), np.float16)
    rng = np.arange(P)
    for e in range(E):
        wm[rng, e, rng] = m[e][cidx]
    wm[rng, E, rng] = 1.0
    wm[rng, E + 1, rng] = -2.0

    in_maps = []
    for b in range(NCORES):
        padded = np.pad(inp[b], ((1, 1), (1, 1), (0, 0)))  # [226,226,32]
        # quarters: q needs padded rows [56q, 56q+58)
        qs = np.stack(
            [padded[RQ * q : RQ * q + RA] for q in range(Q)], axis=0
        )  # [4,58,226,32]
        x_dev = np.ascontiguousarray(
            qs.transpose(0, 3, 1, 2).reshape(P, RA, WP)
        )
        in_maps.append(
            {
                "xh": x_dev.astype(np.float16),
                "pv": pv,
                "wm": wm,
            }
        )
    return in_maps


def _host_unpack(results):
    out = np.empty((B, H, W, C), np.float32)
    for b in range(NCORES):
        y = results[b]["y"].astype(np.float32).reshape(Q, C, RQ, W)
        out[b] = y.transpose(0, 2, 3, 1).reshape(H, W, C)
    return out


LAST_PROFILE = {}


def _install_ntff_shim():
    """antenv.axon_hooks is missing in this image; synthesize it so
    run_bass_kernel_spmd(trace=True) can capture NTFF profiles."""
    import contextlib
    import ctypes
    import types

    if "antenv.axon_hooks" in sys.modules:
        return
    so_path = "/opt/axon/libaxon_pjrt.so"
    try:
        lib = ctypes.CDLL(so_path)
    except OSError:
        return
    if not hasattr(lib, "axon_start_nrt_profile"):
        return
    lib.axon_start_nrt_profile.argtypes = [
        ctypes.POINTER(ctypes.c_int64),
        ctypes.c_size_t,
    ]
    lib.axon_start_nrt_profile.restype = ctypes.c_int64
    lib.axon_stop_nrt_profile.argtypes = [ctypes.c_char_p]
    lib.axon_stop_nrt_profile.restype = ctypes.c_int64

    @contextlib.contextmanager
    def _hook(output_dir, device_ids):
        import jax

        jax.devices()
        if device_ids:
            ids = (ctypes.c_int64 * len(device_ids))(*device_ids)
            rc = lib.axon_start_nrt_profile(ids, len(device_ids))
        else:
            rc = lib.axon_start_nrt_profile(None, 0)
        if rc != 0:
            raise RuntimeError(f"axon_start_nrt_profile rc={rc}")
        try:
            yield
        finally:
            n = lib.axon_stop_nrt_profile(str(output_dir).encode())
            if n < 0:
                raise RuntimeError(f"axon_stop_nrt_profile rc={n}")
            print(f"ntff profile: {n} file(s) written to {output_dir}")

    mod = types.ModuleType("antenv.axon_hooks")
    mod._hook = _hook
    mod.get_axon_ntff_profile_hook = lambda: mod._hook
    mod.set_axon_ntff_profile_hook = lambda h: setattr(mod, "_hook", h)
    sys.modules["antenv.axon_hooks"] = mod


def kernel(inp, kernel, mask):
    if "nc" not in _CACHE:
        _CACHE["nc"] = _build_program()
    nc = _CACHE["nc"]

    in_maps = _host_pack(inp, kernel, mask)
    trace = bool(int(os.environ.get("FM_TRACE", "0")))
    if trace:
        _install_ntff_shim()
    res = run_bass_kernel_spmd(
        nc, in_maps, core_ids=list(range(NCORES)), trace=trace
    )
    LAST_PROFILE["exec_time_ns"] = res.exec_time_ns
    LAST_PROFILE["mean_exec_time_ns"] = res.mean_exec_time_ns
    return _host_unpack(res.results)


# revision 21
# speedup vs baseline: 1.0000x; 1.0000x over previous
"""Trainium2 Bass kernel for nn_FactorMask (9-tap masked-stencil op).

Contract: kernel(**inputs) takes FULL inputs (inp [8,224,224,32] f32,
kernel [9,1,1,1,32], mask [9,1,1,1,32]) and returns the FULL output
[8,224,224,32] f32. Internally: batch-parallel across 8 NeuronCores
(1 image per core), host-side repack to a channels-on-partition layout,
Bass/Tile kernel per core, host-side gather.

Layout per core: xh [128, 58, 226] fp16 where partition p = 32*q + c
(q = H-quarter 0..3, c = channel), rows = 58 padded rows of that quarter
(56 output rows + 1 halo row each side), cols = 226 zero-padded W.
All 9 stencil taps are then pure free-dim AP offsets.

Math (per pixel/channel): A_e = |m_e x_e - k_e|, mu = M/9 - kbar with
M = sum m_e x_e, norm1 = sum A_e, S = sum min(A_e, mu),
    var = norm1 + 9 mu - 2 S
    out = (1 - var/9)(1 - norm1/9) = W * (W + kbar - F/9)
with W = (9 - norm1)/9 and F = M - 2S.

v4 structure (per segment-pair of 2x2 rows):
- PE bank MF: 9 mean-conv matmuls (diag m_e) -> Act folds mu; later the
  9 min-field matmuls (-2I) CONTINUE accumulating into the same bank
  (start=False) so F = M - 2S costs no extra pass.
- PE bank N1: identity matmuls over A fields -> Act folds W = 1 - N1/9.
- Fused final: one custom DVE op out = Src0*((Src0+C0) - Src1*C1) with
  Src0 = W (SBUF fp16), Src1 = F (PSUM), C0 = kbar, C1 = 1/9 -- replaces
  the old STT + 2xTT chain (DVE instructions can read only ONE PSUM
  operand, so W must come via the Act fold).
"""

import os
import sys

for _p in ("/opt/trn_rl_repo", "/opt/pypackages"):
    if _p not in sys.path:
        sys.path.insert(0, _p)

import numpy as np

import concourse.bacc as bacc
import concourse.mybir as mybir
import concourse.tile as tile
from concourse.bass_utils import run_bass_kernel_spmd

# ---- problem constants (hardcoded per the task spec) ----
B, H, W, C = 8, 224, 224, 32
E = 9  # taps
NCORES = 8
Q = 4  # H-quarters per image -> 4*32 = 128 partitions
RQ = H // Q  # 56 output rows per quarter
RA = RQ + 2  # 58 rows incl halo
WP = W + 2  # 226 padded cols
P = 128

# tap order must match reference: element 0 = center, then (y,x) raster
# order skipping center, with shift (dy,dx) = (y-1, x-1)
TAPS = [(0, 0)] + [
    (dy, dx) for dy in (-1, 0, 1) for dx in (-1, 0, 1) if not (dy == 0 and dx == 0)
]

# ---- tunables ----
SEG = 2  # rows per matmul segment (2*224 = 448 <= 512)
NDVE = int(os.environ.get("FM_NDVE", "2"))  # taps whose A is made on DVE
NPAIR_A = int(os.environ.get("FM_NPAIR_A", "1"))  # DVE pre-sum pairs (norm1)
NPAIR_S = int(os.environ.get("FM_NPAIR_S", "4"))  # DVE pre-sum pairs (S)
F32 = mybir.dt.float32
HDT = mybir.dt.float16
U16 = mybir.dt.uint16

_CACHE = {}


def _register_fm_final():
    import concourse.dve_ops as DOPS
    from concourse.dve_spec import Spec, Src0, Src1, C0, C1, lower

    name = "FM_FINAL_ANT"
    if name in DOPS._SUB_OPCODE_FOR_NAME:
        for op in DOPS.OPS:
            if op.name == name:
                return op
    spec = Spec(
        body=Src0 * ((Src0 + C0) - Src1 * C1),
        reference=lambda in0, in1, s0, s1, imm2: (
            in0 * ((in0 + s0) - in1 * s1)
        ),
    )
    row = max(DOPS._SUB_OPCODE_FOR_NAME.values()) + 1
    assert row < 0x20
    DOPS._SUB_OPCODE_FOR_NAME[name] = row
    shas = {}
    for ver in ("v3", "v4"):
        shas[ver] = DOPS.DveOpSpec(
            name=name, opcode=row, uops=lower(spec, ver=ver), rd1_en=True
        ).sha(ver)
    op = DOPS.DveOp(name, spec, subdim=False, uops_sha=shas)
    DOPS.OPS.append(op)
    DOPS.CUSTOM_DVE_SPECS[name] = spec
    return op


def _build_program():
    nc = bacc.Bacc(
        "TRN2", target_bir_lowering=False, debug=False, num_devices=NCORES
    )
    xh_d = nc.dram_tensor("xh", [P, RA, WP], HDT, kind="ExternalInput").ap()
    # pv columns: [0:9]=m_e scale, [9:18]=-k_e bias, [18]=-kbar, [19]=+kbar
    pv_d = nc.dram_tensor("pv", [P, 2 * E + 2], F32, kind="ExternalInput").ap()
    # wm[:, e, :] = diag(m_e) e<9 ; wm[:, 9, :] = I ; wm[:, 10, :] = -2I
    wm_d = nc.dram_tensor("wm", [P, E + 2, P], HDT, kind="ExternalInput").ap()
    y_d = nc.dram_tensor("y", [P, RQ, W], HDT, kind="ExternalOutput").ap()

    fm_final = _register_fm_final()
    with tile.TileContext(nc) as tc:
        _emit(tc, nc, xh_d, pv_d, wm_d, y_d, fm_final)
    nc.compile()
    return nc


def _band_sizes():
    sched = os.environ.get("FM_BANDS", "4,8,8,8,8,8,8,4")
    sizes = [int(s) for s in sched.split(",")]
    assert sum(sizes) == RQ and all(s % (2 * SEG) == 0 for s in sizes)
    return sizes


def _emit(tc, nc, xh_d, pv_d, wm_d, y_d, fm_final):
    Abs = mybir.ActivationFunctionType.Abs
    Ident = mybir.ActivationFunctionType.Identity
    Copy = mybir.ActivationFunctionType.Copy
    mult = mybir.AluOpType.mult
    add = mybir.AluOpType.add
    amin = mybir.AluOpType.min
    band_ = mybir.AluOpType.bitwise_and

    def band_cfg(band, nbands):
        # late bands: DVE is the endgame bottleneck (mins/finals with no Act
        # work left) -> fewer DVE pre-sum pairs, taps on Act.
        tail = int(os.environ.get("FM_TAILBANDS", "2"))
        if band >= nbands - tail:
            return (
                int(os.environ.get("FM_TNDVE", "0")),
                int(os.environ.get("FM_TNPAIR_A", "1")),
                int(os.environ.get("FM_TNPAIR_S", "0")),
            )
        return (NDVE, NPAIR_A, NPAIR_S)

    with (
        tc.tile_pool(name="const", bufs=1) as cpool,
        tc.tile_pool(name="xin", bufs=int(os.environ.get("FM_XBUFS", "3"))) as xpool,
        tc.tile_pool(name="work", bufs=2) as wpool,
        tc.tile_pool(name="mf", bufs=int(os.environ.get("FM_MFBUFS","6")), space="PSUM") as mfpool,
        tc.tile_pool(name="n1", bufs=int(os.environ.get("FM_N1BUFS","2")), space="PSUM") as n1pool,
    ):
        # Startup DMA issue order: first matmul gates on wm + band-0 xh, so
        # those two go first; pv (tiny, needed by Act) issues third.
        sizes = _band_sizes()
        # spread startup DMAs across queues so they trigger in parallel as
        # soon as each engine's preamble finishes
        xbh0 = xpool.tile([P, sizes[0] + 2, WP], HDT, tag="xbh")
        nc.sync.dma_start(xbh0[:], xh_d[:, 0 : sizes[0] + 2, :])
        wm = cpool.tile([P, E + 2, P], HDT)
        nc.gpsimd.dma_start(wm[:], wm_d[:])
        pv = cpool.tile([P, 2 * E + 2], F32)
        nc.scalar.dma_start(pv[:], pv_d[:])

        # PE p-state warmup: the Tensor engine ramps 0.65->2.4GHz over ~3us
        # of continuous work. Run throwaway matmuls on a memset tile during
        # the startup DMA wait so real matmuls start at full clock.
        nwarm = int(os.environ.get("FM_WARM", "16"))
        if nwarm:
            warm = cpool.tile([P, 512], HDT)
            nc.vector.memset(warm[:], 0.5)
            wp = n1pool.tile([P, SEG * W], F32, tag="n1")
            for _ in range(nwarm):
                nc.tensor.matmul(
                    wp[:], warm[:, 0:P], warm[:, 0 : SEG * W], start=True, stop=True
                )

        r0 = 0
        for band, rb in enumerate(sizes):
            ndve_b, npa_b, nps_b = band_cfg(band, len(sizes))
            a_pairs = [(1 + 2 * i, 2 + 2 * i) for i in range(npa_b)]
            s_pairs = [(1 + 2 * i, 2 + 2 * i) for i in range(nps_b)]
            a_paired = {t for p in a_pairs for t in p}
            s_paired = {t for p in s_pairs for t in p}
            nseg = rb // SEG
            npair = nseg // 2
            if band == 0:
                xbh = xbh0
            else:
                xbh = xpool.tile([P, rb + 2, WP], HDT, tag="xbh")
                nc.sync.dma_start(xbh[:], xh_d[:, r0 : r0 + rb + 2, :])

            # A_e = |m_e * x - k_e| on the tap's shifted window. The last
            # NDVE taps are made on DVE (two 4x tensor_scalar ops).
            A = [None] * E

            def emit_a(e):
                dy, dx = TAPS[e]
                a = wpool.tile(
                    [P, rb, W], HDT, tag=f"A{e}",
                    bufs=3 if e < int(os.environ.get("FM_ADEEP", "7")) else 2,
                )
                win = xbh[:, 1 + dy : 1 + dy + rb, 1 + dx : 1 + dx + W]
                if e < E - ndve_b:
                    nc.scalar.activation(
                        a[:],
                        win,
                        Abs,
                        bias=pv[:, E + e : E + e + 1],
                        scale=pv[:, e : e + 1],
                    )
                else:
                    nc.vector.tensor_scalar(
                        a[:],
                        win,
                        pv[:, e : e + 1],
                        pv[:, E + e : E + e + 1],
                        mult,
                        add,
                    )
                    nc.vector.tensor_scalar(
                        a[:].bitcast(U16), a[:].bitcast(U16), 0x7FFF, None, band_
                    )
                A[e] = a

            # First Act A-fields go ahead of the mu-folds in the Act queue:
            # the mu-folds wait on PE chains, and an in-order Act queue would
            # otherwise idle at band start with ready A-work behind them.
            n_early = int(os.environ.get("FM_AEARLY", "3"))
            for e in range(min(n_early, E - ndve_b)):
                emit_a(e)

            # mean conv per segment into 1-bank MF tiles (deep pipelining);
            # mu = MF/9 - kbar folded per segment on Act.
            mean = wpool.tile(
                [P, rb, W], HDT, tag="mean",
                bufs=int(os.environ.get("FM_MBUFS", "2")),
            )
            mfs = []
            for s in range(nseg):
                mf = mfpool.tile([P, SEG * W], F32, tag="mf")
                mfs.append(mf)
                i0 = s * SEG
                for e, (dy, dx) in enumerate(TAPS):
                    rhs = xbh[
                        :, i0 + 1 + dy : i0 + 1 + dy + SEG, 1 + dx : 1 + dx + W
                    ]
                    nc.tensor.matmul(
                        mf[:],
                        wm[:, e, :],
                        rhs,
                        start=(e == 0),
                        stop=(e == E - 1),
                    )
                nc.scalar.activation(
                    mean[:, i0 : i0 + SEG, :],
                    mf[:],
                    Ident,
                    bias=pv[:, 2 * E : 2 * E + 1],
                    scale=1.0 / E,
                )

            for e in range(E):
                if A[e] is None:
                    emit_a(e)

            # optional DVE pre-sums of A pairs (frees PE passes)
            APs = {}
            for i, (t0, t1) in enumerate(a_pairs):
                pa = wpool.tile([P, rb, W], HDT, tag=f"PA{i}")
                nc.vector.tensor_tensor(pa[:], A[t0][:], A[t1][:], add)
                APs[(t0, t1)] = pa
            norm_fields = [A[0]] + [APs[p] for p in a_pairs] + [
                A[e] for e in range(1, E) if e not in a_paired
            ]

            # norm1 per segment (1-bank tiles, fine-grained pipelining);
            # W = (9 - norm1)/9 folded per segment on Act.
            SW = SEG * W
            w9 = wpool.tile([P, rb, W], HDT, tag="w9")
            for s in range(nseg):
                n1 = n1pool.tile([P, SW], F32, tag="n1")
                i0 = s * SEG
                for jf, f in enumerate(norm_fields):
                    nc.tensor.matmul(
                        n1[:],
                        wm[:, E, :],
                        f[:, i0 : i0 + SEG, :],
                        start=(jf == 0),
                        stop=(jf == len(norm_fields) - 1),
                    )
                nc.scalar.activation(
                    w9[:, i0 : i0 + SEG, :],
                    n1[:],
                    Copy,
                    bias=1.0,
                    scale=-1.0 / E,
                )

            # min-fields sm_e = min(A_e, mean)  (one 2x TT per tap).
            # FM_INPLACE=1 overwrites A_e (saves SBUF for bigger bands; the
            # TT then waits for the N1 matmuls that consume A_e).
            inplace = bool(int(os.environ.get("FM_INPLACE", "0")))
            sm = []
            for e in range(E):
                if inplace:
                    t = A[e]
                else:
                    t = wpool.tile(
                        [P, rb, W],
                        HDT,
                        tag=f"sm{e}",
                        bufs=int(os.environ.get("FM_SMBUFS", "2")),
                    )
                nc.vector.tensor_tensor(t[:], A[e][:], mean[:], amin)
                sm.append(t)

            # in-place pair pre-sums: sm[t0] += sm[t1]
            for t0, t1 in s_pairs:
                nc.vector.tensor_tensor(sm[t0][:], sm[t0][:], sm[t1][:], add)
            s_fields = [sm[0]] + [sm[t0] for t0, _ in s_pairs] + [
                sm[e] for e in range(1, E) if e not in s_paired
            ]

            # F = M - 2S: min-field matmuls (-2I) continue accumulating into
            # the closed MF group (start=False); then the fused final per
            # segment: out = W * ((W + kbar) - F/9)  (one custom DVE op).
            ob = wpool.tile([P, rb, W], HDT, tag="ob")
            for s in range(nseg):
                mf = mfs[s]
                i0 = s * SEG
                for jf, f in enumerate(s_fields):
                    nc.tensor.matmul(
                        mf[:],
                        wm[:, E + 1, :],
                        f[:, i0 : i0 + SEG, :],
                        start=False,
                        stop=(jf == len(s_fields) - 1),
                        skip_group_check=True,
                    )
                i1 = i0 + SEG
                nc.vector._custom_dve(
                    fm_final,
                    out=ob[:, i0:i1, :],
                    in0=w9[:, i0:i1, :],
                    in1=mf[:],
                    s0=pv[:, 2 * E + 1 : 2 * E + 2],
                    s1=1.0 / E,
                )
            if band == len(sizes) - 1 and int(os.environ.get("FM_SPLITOUT", "1")):
                # last band: per-segment out-DMA so the first segment's store
                # overlaps the last segment's final
                for s2 in range(nseg):
                    i0 = s2 * SEG
                    nc.sync.dma_start(
                        y_d[:, r0 + i0 : r0 + i0 + SEG, :], ob[:, i0 : i0 + SEG, :]
                    )
            else:
                nc.sync.dma_start(y_d[:, r0 : r0 + rb, :], ob[:])
            r0 += rb


def _host_pack(inp, kern, mask):
    """Build per-core input maps."""
    inp = np.ascontiguousarray(inp, dtype=np.float32)
    kern = np.asarray(kern, dtype=np.float32).reshape(E, C)
    mask = np.asarray(mask, dtype=np.float32).reshape(E, C)

    m = np.abs(mask) / (np.abs(mask).max() + np.float32(1e-6))  # [E,C]
    kbar = kern.mean(axis=0)  # [C]

    cidx = np.arange(P) % C
    pv = np.empty((P, 2 * E + 2), np.float32)
    for e in range(E):
        pv[:, e] = m[e][cidx]
        pv[:, E + e] = -kern[e][cidx]
    pv[:, 2 * E] = -kbar[cidx]
    pv[:, 2 * E + 1] = kbar[cidx]

    wm = np.zeros((P, E + 2, P), np.float16)
    rng = np.arange(P)
    for e in range(E):
        wm[rng, e, rng] = m[e][cidx]
    wm[rng, E, rng] = 1.0
    wm[rng, E + 1, rng] = -2.0

    in_maps = []
    for b in range(NCORES):
        padded = np.pad(inp[b], ((1, 1), (1, 1), (0, 0)))  # [226,226,32]
        # quarters: q needs padded rows [56q, 56q+58)
        qs = np.stack(
            [padded[RQ * q : RQ * q + RA] for q in range(Q)], axis=0
        )  # [4,58,226,32]
        x_dev = np.ascontiguousarray(
            qs.transpose(0, 3, 1, 2).reshape(P, RA, WP)
        )
        in_maps.append(
            {
                "xh": x_dev.astype(np.float16),
                "pv": pv,
                "wm": wm,
            }
        )
    return in_maps


def _host_unpack(results):
    out = np.empty((B, H, W, C), np.float32)
    for b in range(NCORES):
        y = results[b]["y"].astype(np.float32).reshape(Q, C, RQ, W)
        out[b] = y.transpose(0, 2, 3, 1).reshape(H, W, C)
    return out


LAST_PROFILE = {}


def _install_ntff_shim():
    """antenv.axon_hooks is missing in this image; synthesize it so
    run_bass_kernel_spmd(trace=True) can capture NTFF profiles."""
    import contextlib
    import ctypes
    import types

    if "antenv.axon_hooks" in sys.modules:
        return
    so_path = "/opt/axon/libaxon_pjrt.so"
    try:
        lib = ctypes.CDLL(so_path)
    except OSError:
        return
    if not hasattr(lib, "axon_start_nrt_profile"):
        return
    lib.axon_start_nrt_profile.argtypes = [
        ctypes.POINTER(ctypes.c_int64),
        ctypes.c_size_t,
    ]
    lib.axon_start_nrt_profile.restype = ctypes.c_int64
    lib.axon_stop_nrt_profile.argtypes = [ctypes.c_char_p]
    lib.axon_stop_nrt_profile.restype = ctypes.c_int64

    @contextlib.contextmanager
    def _hook(output_dir, device_ids):
        import jax

        jax.devices()
        if device_ids:
            ids = (ctypes.c_int64 * len(device_ids))(*device_ids)
            rc = lib.axon_start_nrt_profile(ids, len(device_ids))
        else:
            rc = lib.axon_start_nrt_profile(None, 0)
        if rc != 0:
            raise RuntimeError(f"axon_start_nrt_profile rc={rc}")
        try:
            yield
        finally:
            n = lib.axon_stop_nrt_profile(str(output_dir).encode())
            if n < 0:
                raise RuntimeError(f"axon_stop_nrt_profile rc={n}")
            print(f"ntff profile: {n} file(s) written to {output_dir}")

    mod = types.ModuleType("antenv.axon_hooks")
    mod._hook = _hook
    mod.get_axon_ntff_profile_hook = lambda: mod._hook
    mod.set_axon_ntff_profile_hook = lambda h: setattr(mod, "_hook", h)
    sys.modules["antenv.axon_hooks"] = mod


def kernel(inp, kernel, mask):
    if "nc" not in _CACHE:
        _CACHE["nc"] = _build_program()
    nc = _CACHE["nc"]

    in_maps = _host_pack(inp, kernel, mask)
    trace = bool(int(os.environ.get("FM_TRACE", "0")))
    if trace:
        _install_ntff_shim()
    res = run_bass_kernel_spmd(
        nc, in_maps, core_ids=list(range(NCORES)), trace=trace
    )
    LAST_PROFILE["exec_time_ns"] = res.exec_time_ns
    LAST_PROFILE["mean_exec_time_ns"] = res.mean_exec_time_ns
    return _host_unpack(res.results)
